# revision 24
# baseline (speedup 1.0000x reference)
import sys

import numpy as np

for _p in ("/opt/trn_rl_repo",):
    if _p not in sys.path:
        sys.path.insert(0, _p)

B = 4096
M = 8192
EMB = 64
K = 4
TAU = 0.3
NCORES = 8
BLOC = B // NCORES  # 512 batch rows per core
P = 128             # batch rows per tile
NBT = BLOC // P     # 4 tiles per core

_CACHE = {}


def _build(debug=False, iters=1, no_gather=False, no_smalls=False, no_topk=False,
           no_combine=False, no_squares=False, dt16=True, big_eng="sync",
           abufs=2, fullrow=True, cm=4096, tt_comb=True, ser=False):
    from contextlib import ExitStack

    import concourse.bacc as bacc
    import concourse.bass as bass
    import concourse.mybir as mybir
    import concourse.tile as tile
    from concourse.masks import make_identity

    f32 = mybir.dt.float32
    u32 = mybir.dt.uint32
    d16 = mybir.dt.bfloat16 if dt16 else f32
    AF = mybir.ActivationFunctionType
    OP = mybir.AluOpType
    AX = mybir.AxisListType

    CM = M if fullrow else cm   # anchors per chunk
    NCH = M // CM
    NTI = NBT * 2               # branch-tiles per iteration

    nc = bacc.Bacc()
    nodes_h = nc.declare_dram_parameter("nodes", [BLOC, 2, 2], f32, isOutput=False)
    # anchors declared flat: [row, interleaved (m c)] so the big loads are
    # plain 2D slices (no rearrange APs in the DMA descriptors)
    ancS_h = nc.declare_dram_parameter("ancS", [BLOC, 2 * M], f32, isOutput=False)
    ancL_h = nc.declare_dram_parameter("ancL", [BLOC, 2 * M], f32, isOutput=False)
    W1_h = nc.declare_dram_parameter("W1", [EMB, 2], f32, isOutput=False)
    b1_h = nc.declare_dram_parameter("b1", [EMB], f32, isOutput=False)
    W2_h = nc.declare_dram_parameter("W2", [EMB, EMB], f32, isOutput=False)
    b2_h = nc.declare_dram_parameter("b2", [EMB], f32, isOutput=False)
    rowbase_h = nc.declare_dram_parameter("rowbase", [BLOC, 1], u32, isOutput=False)
    out_h = nc.declare_dram_parameter("out", [BLOC, 2 * EMB], f32, isOutput=True)

    with ExitStack() as ctx:
        tc = ctx.enter_context(tile.TileContext(nc))
        const = ctx.enter_context(tc.tile_pool(name="const", bufs=1))
        a_pool = ctx.enter_context(tc.tile_pool(name="a", bufs=abufs))
        sq_pool = ctx.enter_context(tc.tile_pool(name="sq", bufs=2))
        mneg_pool = ctx.enter_context(tc.tile_pool(name="mneg", bufs=2))
        keep = ctx.enter_context(tc.tile_pool(name="keep", bufs=2))
        small = ctx.enter_context(tc.tile_pool(name="small", bufs=2))
        mlp = ctx.enter_context(tc.tile_pool(name="mlp", bufs=2))
        psum = ctx.enter_context(tc.tile_pool(name="psum", bufs=1, space="PSUM"))

        ident = const.tile([P, P], f32)
        make_identity(nc, ident[:])

        # Warm-up Gelu: anchors the ACT table chooser on gelu_and_others
        # (gelu/square/tanh/copy) so the kernel needs a single table load.
        dummy = const.tile([1, 1], f32)
        nc.scalar.activation(dummy[:], dummy[:], AF.Gelu, bias=0.0, scale=1.0)

        w1t = const.tile([2, EMB], f32)  # w1t[c, e] = W1[e, c]
        nc.sync.dma_start(out=w1t[:], in_=W1_h[:].rearrange("e c -> c e"))
        w2t = const.tile([EMB, EMB], f32)  # w2t[e, f] = W2[f, e]
        nc.sync.dma_start(out=w2t[:], in_=W2_h[:].rearrange("f e -> e f"))
        b1c = const.tile([EMB, 1], f32)
        nc.sync.dma_start(out=b1c[:], in_=b1_h[:].rearrange("(e u) -> e u", u=1))
        b2c = const.tile([EMB, 1], f32)
        nc.sync.dma_start(out=b2c[:], in_=b2_h[:].rearrange("(e u) -> e u", u=1))

        big_engs = {
            "sync": [nc.sync],
            "gpsimd": [nc.gpsimd],
            "alt": [nc.sync, nc.gpsimd],
        }[big_eng]

        if ser:
            # serialization token: iteration i+1's first big DMA is gated on
            # iteration i's last tail op, so an iters>1 loop approximates
            # independent single-shot executions for timing.
            serp = ctx.enter_context(tc.tile_pool(name="serp", bufs=1))
            ser_t = serp.tile([1, 1], f32)
            nc.vector.memset(ser_t[:], 0.0)

        def _body():
            vals_all = keep.tile([P, 8 * NTI], d16)
            idx_all = keep.tile([P, 8 * NTI], u32)

            for bt in range(NBT):
                rows = slice(bt * P, (bt + 1) * P)

                # tiny per-tile loads ride the scalar (ACT) HWDGE ring so the
                # big-load stream on the sync ring stays clean
                nodes_t = small.tile([P, 4], f32)
                nc.scalar.dma_start(
                    out=nodes_t[:],
                    in_=nodes_h[rows, :, :].rearrange("p a c -> p (a c)"),
                )
                negn = small.tile([P, 4], f32)
                nc.vector.tensor_scalar(
                    out=negn[:], in0=nodes_t[:], scalar1=-1.0, scalar2=None,
                    op0=OP.mult,
                )
                rowbase = small.tile([P, 1], u32)
                nc.scalar.dma_start(out=rowbase[:], in_=rowbase_h[rows, :])

                for br in range(2):
                    ti = bt * 2 + br
                    anc_h = ancS_h if br == 0 else ancL_h
                    mneg = mneg_pool.tile([P, M], d16)
                    for chk in range(NCH):
                        a_t = a_pool.tile([P, 2 * CM], f32)
                        if ser and ti == 0 and chk == 0:
                            # read the token (RAW on prev iteration's write),
                            # then the DMA's WAW on a_t gates the whole stream
                            nc.vector.tensor_scalar(
                                out=a_t[0:1, 0:1], in0=ser_t[:], scalar1=0.0,
                                scalar2=None, op0=OP.mult,
                            )
                        eng = big_engs[(ti * NCH + chk) % len(big_engs)]
                        eng.dma_start(
                            out=a_t[:],
                            in_=anc_h[rows, 2 * chk * CM:2 * (chk + 1) * CM],
                        )
                        av = a_t[:].rearrange("p (m c) -> p m c", c=2)
                        msl = mneg[:, chk * CM:(chk + 1) * CM]
                        v2 = sq_pool.tile([P, CM], d16)
                        if no_squares:
                            continue
                        # u^2 straight into the mneg slice; v^2 to scratch
                        nc.scalar.activation(
                            msl, av[:, :, 0], AF.Square,
                            bias=negn[:, 2 * br:2 * br + 1], scale=1.0,
                        )
                        nc.scalar.activation(
                            v2[:], av[:, :, 1], AF.Square,
                            bias=negn[:, 2 * br + 1:2 * br + 2], scale=1.0,
                        )
                        if no_combine:
                            continue
                        if tt_comb and dt16:
                            nc.vector.tensor_tensor(
                                out=msl, in0=v2[:], in1=msl, op=OP.add,
                            )
                        else:
                            # mneg = (v2 * -1) - u2 = -d2, one fused DVE pass
                            nc.vector.scalar_tensor_tensor(
                                out=msl, in0=v2[:], scalar=-1.0, in1=msl,
                                op0=OP.mult, op1=OP.subtract,
                            )

                    if no_topk or no_combine or no_squares:
                        continue
                    if tt_comb and dt16:
                        # mneg currently holds +d2: negate in one 4x pass
                        nc.vector.tensor_scalar(
                            out=mneg[:], in0=mneg[:], scalar1=-1.0, scalar2=None,
                            op0=OP.mult,
                        )
                    vals8 = vals_all[:, 8 * ti:8 * ti + 8]
                    idx8 = idx_all[:, 8 * ti:8 * ti + 8]
                    nc.vector.max(out=vals8, in_=mneg[:])
                    nc.vector.max_index(out=idx8, in_max=vals8, in_values=mneg[:])
                    if no_smalls:
                        continue

                    gidx = small.tile([P, K], u32)
                    nc.vector.tensor_tensor(
                        out=gidx[:], in0=idx8[:, 0:K],
                        in1=rowbase[:].to_broadcast([P, K]), op=OP.add,
                    )

                    comb = small.tile([P, 2 * K], f32)
                    if no_gather:
                        nc.vector.memset(comb[:], 0.5)
                    else:
                        for k in range(K):
                            nc.gpsimd.indirect_dma_start(
                                out=comb[:, 2 * k:2 * k + 2],
                                out_offset=None,
                                in_=anc_h[:].rearrange("b (m c) -> (b m) c", c=2),
                                in_offset=bass.IndirectOffsetOnAxis(
                                    ap=gidx[:, k:k + 1], axis=0),
                            )

                    # w = softmax(d2_topk / tau); vals8[:, :K] = -d2 (descending)
                    t4 = small.tile([P, K], f32)
                    nc.vector.tensor_scalar(
                        out=t4[:], in0=vals8[:, 0:K], scalar1=float(-1.0 / TAU),
                        scalar2=None, op0=OP.mult,
                    )
                    nrmaxh = small.tile([P, 1], f32)
                    nc.vector.tensor_scalar(
                        out=nrmaxh[:], in0=vals8[:, K - 1:K], scalar1=float(0.5 / TAU),
                        scalar2=None, op0=OP.mult,
                    )
                    # exp(x) = 2/(1 - tanh(x/2)) - 1 with x = t4 - max(t4);
                    # keeps ACT inside the gelu_and_others table set.
                    th = small.tile([P, K], f32)
                    nc.scalar.activation(th[:], t4[:], AF.Tanh, bias=nrmaxh[:, 0:1], scale=0.5)
                    denom = small.tile([P, K], f32)
                    nc.vector.tensor_scalar(
                        out=denom[:], in0=th[:], scalar1=-1.0, scalar2=1.0,
                        op0=OP.mult, op1=OP.add,
                    )
                    rden = small.tile([P, K], f32)
                    nc.vector.reciprocal(rden[:], denom[:])
                    e4 = small.tile([P, K], f32)
                    nc.vector.tensor_scalar(
                        out=e4[:], in0=rden[:], scalar1=2.0, scalar2=-1.0,
                        op0=OP.mult, op1=OP.add,
                    )
                    ssum = small.tile([P, 1], f32)
                    nc.vector.reduce_sum(out=ssum[:], in_=e4[:], axis=AX.X)
                    rinv = small.tile([P, 1], f32)
                    nc.vector.reciprocal(rinv[:], ssum[:])
                    wnorm = small.tile([P, K], f32)
                    nc.vector.tensor_scalar(
                        out=wnorm[:], in0=e4[:], scalar1=rinv[:, 0:1],
                        scalar2=None, op0=OP.mult,
                    )

                    # [P, 2] slices -> [2, P] each, packed into [2, K*P]
                    tp_ps = psum.tile([2, K * P], f32)
                    for k in range(K):
                        nc.tensor.transpose(
                            out=tp_ps[:, k * P:(k + 1) * P],
                            in_=comb[:, 2 * k:2 * k + 2], identity=ident[:],
                        )
                    rhs_all = mlp.tile([2, K * P], f32)
                    nc.vector.tensor_copy(rhs_all[:], tp_ps[:])

                    hp = psum.tile([EMB, K * P], f32)
                    nc.tensor.matmul(
                        out=hp[:], lhsT=w1t[:], rhs=rhs_all[:], start=True, stop=True
                    )
                    h1 = mlp.tile([EMB, K * P], f32)
                    nc.scalar.activation(h1[:], hp[:], AF.Gelu, bias=b1c[:, 0:1], scale=1.0)
                    h2p = psum.tile([EMB, K * P], f32)
                    nc.tensor.matmul(
                        out=h2p[:], lhsT=w2t[:], rhs=h1[:], start=True, stop=True
                    )
                    h2 = mlp.tile([EMB, K * P], f32)
                    nc.scalar.activation(h2[:], h2p[:], AF.Gelu, bias=b2c[:, 0:1], scale=1.0)

                    # weighted sum over neighbors, back to b-on-partitions layout
                    osb = small.tile([P, EMB], f32)
                    nc.vector.memset(osb[:], 0.0)
                    for k in range(K):
                        h2tp = psum.tile([P, EMB], f32)
                        nc.tensor.transpose(
                            out=h2tp[:], in_=h2[:, k * P:(k + 1) * P],
                            identity=ident[0:EMB, 0:EMB],
                        )
                        nc.vector.scalar_tensor_tensor(
                            out=osb[:], in0=h2tp[:], scalar=wnorm[:, k:k + 1],
                            in1=osb[:], op0=OP.mult, op1=OP.add,
                        )
                    nc.scalar.dma_start(
                        out=out_h[rows, br * EMB:(br + 1) * EMB], in_=osb[:]
                    )
                    if ser and ti == NTI - 1:
                        nc.vector.tensor_copy(ser_t[:], osb[0:1, 0:1])

        if iters > 1:
            with tc.For_i(0, iters, 1):
                _body()
        else:
            _body()
    return nc


def _get_nc(debug=False):
    key = ("nc", debug)
    if key not in _CACHE:
        nc = _build(debug)
        nc.finalize()  # runs the Bacc passes (event sems, reg alloc, table loads)
        _CACHE[key] = nc
    return _CACHE[key]


def _make_in_maps(inputs):
    nodes = np.asarray(inputs["nodes_2x2"], dtype=np.float32)
    ancS = np.asarray(inputs["ancS"], dtype=np.float32).reshape(B, 2 * M)
    ancL = np.asarray(inputs["ancL"], dtype=np.float32).reshape(B, 2 * M)
    W1 = np.asarray(inputs["W1"], dtype=np.float32)
    b1 = np.asarray(inputs["b1"], dtype=np.float32)
    W2 = np.asarray(inputs["W2"], dtype=np.float32)
    b2 = np.asarray(inputs["b2"], dtype=np.float32)
    rowbase = (np.arange(BLOC, dtype=np.uint32) * np.uint32(M)).reshape(BLOC, 1)
    in_maps = []
    for c in range(NCORES):
        sl = slice(c * BLOC, (c + 1) * BLOC)
        in_maps.append(
            {
                "nodes": np.ascontiguousarray(nodes[sl]),
                "ancS": np.ascontiguousarray(ancS[sl]),
                "ancL": np.ascontiguousarray(ancL[sl]),
                "W1": W1,
                "b1": b1,
                "W2": W2,
                "b2": b2,
                "rowbase": rowbase,
            }
        )
    return in_maps


def _run(in_maps, trace=False, debug=False):
    from concourse.bass_utils import run_bass_kernel_spmd

    nc = _get_nc(debug)
    return run_bass_kernel_spmd(nc, in_maps, core_ids=list(range(NCORES)), trace=trace)


def kernel(**inputs):
    in_maps = _make_in_maps(inputs)
    res = _run(in_maps).results
    out = np.concatenate([res[c]["out"] for c in range(NCORES)], axis=0)
    return out[:, :EMB].copy(), out[:, EMB:].copy()
